# revision 54
# baseline (speedup 1.0000x reference)
"""Trainium2 Bass kernel for a GPT-style transformer block (B=2, T=2048,
C=1024, 16 heads, MLP 4x), sharded across 8 NeuronCores.

Sharding v2: attention is head-sharded (core 4b+j owns batch b, heads
[4j,4j+4) over ALL 2048 tokens -> exact causal tiling, no kv exchange);
the proj partial output (token-major) is summed+resharded by two chunked
bf16 ReduceScatters (tokens [0,1024) and [1024,2048)), each core receiving
256 tokens per chunk; LN2+MLP run token-sharded on the core's 512 tokens.
The RS output arrives token-major and is flipped back to channel-major by
dma_start_transpose (verified: out[p,s,t] = in[t, 128*s+p]).

Host precomputes LN1, folds LN scale/shift and 1/sqrt(D) into weights,
pre-transposes everything. Residual stays f32 on the output path; matmuls
bf16 with f32 PSUM.
"""
import numpy as np
import ml_dtypes

import concourse.bass as bass
import concourse.mybir as mybir
import concourse.tile as tile
import concourse.bacc as bacc
from concourse.bass_utils import run_bass_kernel_spmd

BF = ml_dtypes.bfloat16
P = 128
B, T, C, H, D, F = 2, 2048, 1024, 16, 64, 4096
NCT = C // P          # 8   c-tiles
NFT = F // P          # 32  f-tiles
NKT = T // P          # 16  kv tiles
EPS = 1e-5
f32 = mybir.dt.float32
bf16 = mybir.dt.bfloat16
f8 = mybir.dt.float8e4
F8 = ml_dtypes.float8_e4m3
WS = 64.0
import os as _os
F8Q = _os.environ.get("KF8Q", "1") == "1"   # qkv+proj+attnT in fp8
F8M = _os.environ.get("KF8M", "0") == "1"   # MLP in fp8
_QDT, _QNP, _QS = (f8, F8, WS) if F8Q else (mybir.dt.bfloat16, ml_dtypes.bfloat16, 1.0)
_MDT, _MNP, _MS = (f8, F8, WS) if F8M else (mybir.dt.bfloat16, ml_dtypes.bfloat16, 1.0)
AF = mybir.ActivationFunctionType

_CACHED_NC = None


def _build_nc():
    nc = bacc.Bacc("TRN2", target_bir_lowering=False, debug=False)
    d = {}
    for name, shape, dt in [
        ("gT", [C, T], _QDT),
        ("WqT", [C, 256], _QDT), ("WkT", [C, 256], _QDT), ("WvT", [C, 256], _QDT),
        ("WpT", [256, C], _QDT), ("WupT", [C, F], _MDT), ("WdownT", [F, C], _MDT),
        ("xbT", [C, 512], bf16),
        ("bq", [P, 2], f32), ("bk", [P, 2], f32), ("brep", [P, 256], bf16),
        ("bup", [P, 32], f32), ("bdown", [P, 8], f32),
        ("maskA", [P, 1024], bf16), ("maskB", [P, 1024], bf16),
    ]:
        d[name] = nc.dram_tensor(name, shape, dt, kind="ExternalInput").ap()
    d["OUT"] = nc.dram_tensor("OUT", [C, 512], f32, kind="ExternalOutput").ap()

    with tile.TileContext(nc) as tc:
        _emit(nc, tc, d)
    nc.compile()
    return nc


def _emit(nc, tc, d):
    import os
    from contextlib import ExitStack

    # token chunks: (qg range, col offset in the core's 512, width, rs rows)
    CH = [(0, 2, 0, 128, 512), (2, 4, 128, 128, 512),
          (4, 6, 256, 128, 512), (6, 8, 384, 128, 512)]

    with ExitStack() as ctx:
        # ---------------- long-lived pools ----------------
        cpool = ctx.enter_context(tc.tile_pool(name="cpool", bufs=1))
        wpool = ctx.enter_context(tc.tile_pool(name="wpool", bufs=1))
        lnp = ctx.enter_context(tc.tile_pool(name="lnp", bufs=1))
        wdp = ctx.enter_context(tc.tile_pool(name="wdp", bufs=1))
        dramp = ctx.enter_context(tc.tile_pool(name="dramp", bufs=1, space="DRAM"))

        attnT = cpool.tile([P, 2, T], _QDT, name="attnT")
        xbT = cpool.tile([P, 8, 512], bf16, name="xbT")        # 8KB
        x1T = cpool.tile([P, 8, 512], bf16, name="x1T")        # 8KB
        maskA = cpool.tile([P, 1024], bf16, name="maskA")
        maskB = cpool.tile([P, 1024], bf16, name="maskB")
        bq = cpool.tile([P, 2], f32, name="bq")
        bk = cpool.tile([P, 2], f32, name="bk")
        brep = cpool.tile([P, 256], bf16, name="brep")
        bup = cpool.tile([P, 32], f32, name="bup")
        bdown = cpool.tile([P, 8], f32, name="bdown")
        epsT = cpool.tile([P, 1], f32, name="epsT")
        onesb = cpool.tile([P, P], bf16, name="onesb")

        wup = wpool.tile([P, NCT, F], _MDT, name="wup")
        wdown_a = wdp.tile([P, 16, C], _MDT, name="wdown_a")

        x1g = lnp.tile([P, 8, 256], bf16, name="x1g")
        sqb = lnp.tile([P, 8, 256], bf16, name="sqb")
        g2 = [lnp.tile([P, 8, 128], _MDT, name=f"g2{c}") for c in range(4)]
        mu = lnp.tile([P, 256], f32, name="mu")
        e2 = lnp.tile([P, 256], f32, name="e2")
        musq = lnp.tile([P, 256], f32, name="musq")
        std = lnp.tile([P, 256], f32, name="std")
        ntr = lnp.tile([P, 256], f32, name="ntr")
        ntt = lnp.tile([P, 256], f32, name="ntt")

        rs_in = [dramp.tile([CH[c][4], C], bf16, name=f"rs_in{c}")
                 for c in range(4)]
        rs_out = [dramp.tile([CH[c][4] // 4, C], bf16, name=f"rs_out{c}")
                  for c in range(4)]

        def emit_ln2_head_a(c):
            """transpose + residual add (SP/DVE only, no PE/Act blocking)."""
            _, _, co, w, _ = CH[c]
            csl = slice(co, co + w)
            nc.sync.dma_start_transpose(x1g[:, :, 0:w], rs_out[c][:])
            with nc.allow_low_precision(reason="residual bf16"):
                nc.vector.tensor_add(x1T[:, :, csl], x1g[:, :, 0:w],
                                     xbT[:, :, csl])

        def emit_ln2_head(c, psum_pool):
            """stats + g2 for chunk c (after emit_ln2_head_a)."""
            _, _, co, w, _ = CH[c]
            csl = slice(co, co + w)
            nc.scalar.activation(sqb[:, :, 0:w], x1T[:, :, csl], AF.Square)
            psmu = psum_pool.tile([P, 1024], f32, name=f"psmu{c}", tag="sc")
            pssq = psum_pool.tile([P, 1024], f32, name=f"pssq{c}", tag="sc")
            for ct in range(NCT):
                nc.tensor.matmul(psmu[:, 0:w], onesb[:], x1T[:, ct, csl],
                                 start=(ct == 0), stop=(ct == NCT - 1))
            for ct in range(NCT):
                nc.tensor.matmul(pssq[:, 0:w], onesb[:], sqb[:, ct, 0:w],
                                 start=(ct == 0), stop=(ct == NCT - 1))
            nc.scalar.mul(mu[:, 0:w], psmu[:, 0:w], 1.0 / C)
            nc.scalar.activation(e2[:, 0:w], pssq[:, 0:w], AF.Copy,
                                 bias=0.0, scale=1.0 / C)
            nc.scalar.activation(musq[:, 0:w], mu[:, 0:w], AF.Square)
            nc.vector.tensor_sub(e2[:, 0:w], e2[:, 0:w], musq[:, 0:w])
            nc.vector.tensor_scalar_add(e2[:, 0:w], e2[:, 0:w], EPS)
            # 1/std = rsqrt(e2) on DVE: reciprocal seed + 2 Newton iters
            # (keeps the Act stream swap-free: only exp/gelu table sets)
            with nc.allow_low_precision(reason="ln2 rsqrt newton"):
                nc.vector.reciprocal(ntr[:, 0:w], e2[:, 0:w])
                nc.vector.tensor_scalar(std[:, 0:w], ntr[:, 0:w], 0.71, 0.29,
                                        mybir.AluOpType.mult,
                                        mybir.AluOpType.add)
                for _ in range(2):
                    nc.vector.tensor_mul(ntt[:, 0:w], std[:, 0:w], std[:, 0:w])
                    nc.vector.tensor_mul(ntt[:, 0:w], ntt[:, 0:w], e2[:, 0:w])
                    nc.vector.tensor_scalar(ntt[:, 0:w], ntt[:, 0:w], -0.5, 1.5,
                                            mybir.AluOpType.mult,
                                            mybir.AluOpType.add)
                    nc.vector.tensor_mul(std[:, 0:w], std[:, 0:w], ntt[:, 0:w])
                for ct in range(NCT):
                    nc.vector.tensor_sub(sqb[:, ct, 0:w], x1T[:, ct, csl],
                                         mu[:, 0:w])
                    nc.vector.tensor_mul(g2[c][:, ct, :], sqb[:, ct, 0:w],
                                         std[:, 0:w])

        # =========== phase 1: QKV + attention + proj ===========
        with tc.tile_pool(name="p1", bufs=1) as p1:
            qT = p1.tile([P, 2, T], bf16, name="qT")           # 8KB
            kT = p1.tile([P, 2, T], bf16, name="kT")           # 8KB
            v_aug = p1.tile([P, 4, NKT * 65], bf16, name="v_aug")   # 8.3KB
            wp = p1.tile([P, 2, C], _QDT, name="wp")
            v4 = v_aug[:].rearrange("p h (k e) -> p h k e", e=65)
            wusrc = d["WupT"].rearrange("(ct p) f -> p ct f", p=P)
            wdsrc = d["WdownT"].rearrange("(cf p) o -> p cf o", p=P)

            # ---- QKV projections ----
            with tc.tile_pool(name="gp", bufs=1) as gp, \
                 tc.tile_pool(name="qkps", bufs=3, space="PSUM") as qkps:
                gT = gp.tile([P, NCT, T], _QDT, name="gT")
                wq = gp.tile([P, NCT, 256], _QDT, name="wq")
                wk = gp.tile([P, NCT, 256], _QDT, name="wk")
                wv = gp.tile([P, NCT, 256], _QDT, name="wv")
                gsrc = d["gT"].rearrange("(ct p) t -> p ct t", p=P)
                nc.sync.dma_start(gT[:, :, 0:512], gsrc[:, :, 0:512])
                for w, key in [(wk, "WkT"), (wq, "WqT"), (wv, "WvT")]:
                    nc.sync.dma_start(w[:],
                                      d[key].rearrange("(ct p) o -> p ct o", p=P))
                for t, key in [(bq, "bq"), (bk, "bk"), (brep, "brep"),
                               (bup, "bup"), (bdown, "bdown"),
                               (maskA, "maskA"), (maskB, "maskB")]:
                    nc.sync.dma_start(t[:], d[key])
                for tch in range(1, 4):
                    nc.sync.dma_start(gT[:, :, tch * 512:(tch + 1) * 512],
                                      gsrc[:, :, tch * 512:(tch + 1) * 512])
                nc.vector.memset(epsT[:], EPS)
                nc.vector.memset(onesb[:], 1.0)
                nc.vector.memset(v4[:, :, :, 64:65], 1.0)
                nc.sync.dma_start(wp[:],
                                  d["WpT"].rearrange("(ct p) o -> p ct o", p=P))
                nc.sync.dma_start(xbT[:],
                                  d["xbT"].rearrange("(ot p) t -> p ot t", p=P))
                nc.sync.dma_start(wup[:], wusrc)
                nc.sync.dma_start(wdown_a[:], wdsrc[:, 0:16, :])

                for tch in range(4):
                    tsl = slice(tch * 512, (tch + 1) * 512)
                    for w, dst, b in [(wk, kT, bk), (wq, qT, bq)]:
                        for ot in range(2):
                            pq = qkps.tile([P, 512], f32, name="pq", tag="qk")
                            if F8Q:
                                for c2 in range(NCT // 2):
                                    nc.tensor.matmul(
                                        pq[:], w[:, 2 * c2:2 * c2 + 2,
                                                 ot * P:(ot + 1) * P],
                                        gT[:, 2 * c2:2 * c2 + 2, tsl],
                                        start=(c2 == 0),
                                        stop=(c2 == NCT // 2 - 1),
                                        perf_mode=mybir.MatmulPerfMode.DoubleRow)
                            else:
                                for ct in range(NCT):
                                    nc.tensor.matmul(
                                        pq[:], w[:, ct, ot * P:(ot + 1) * P],
                                        gT[:, ct, tsl],
                                        start=(ct == 0), stop=(ct == NCT - 1))
                            nc.scalar.activation(dst[:, ot, tsl], pq[:],
                                                 AF.Identity,
                                                 bias=b[:, ot:ot + 1],
                                                 scale=1.0 / _QS)
                    for tt in range(4 * tch, 4 * tch + 4):
                        pv = qkps.tile([P, 256], f32, name="pv", tag="qk")
                        if F8Q:
                            for c2 in range(NCT // 2):
                                nc.tensor.matmul(
                                    pv[:], gT[:, 2 * c2:2 * c2 + 2,
                                              tt * P:(tt + 1) * P],
                                    wv[:, 2 * c2:2 * c2 + 2, :],
                                    start=(c2 == 0), stop=(c2 == NCT // 2 - 1),
                                    perf_mode=mybir.MatmulPerfMode.DoubleRow)
                            vsc = gp.tile([P, 256], f32, name="vsc", tag="vsc",
                                          bufs=2)
                            nc.vector.tensor_scalar_mul(vsc[:], pv[:], 1.0 / WS)
                            pvs = vsc
                        else:
                            for ct in range(NCT):
                                nc.tensor.matmul(
                                    pv[:], gT[:, ct, tt * P:(tt + 1) * P],
                                    wv[:, ct, :],
                                    start=(ct == 0), stop=(ct == NCT - 1))
                            pvs = pv
                        nc.vector.tensor_add(
                            v4[:, :, tt, 0:64],
                            pvs[:].rearrange("p (h dd) -> p h dd", dd=64),
                            brep[:].rearrange("p (h dd) -> p h dd", dd=64))

            # ---- attention (software-pipelined) + proj chunks ----
            with tc.tile_pool(name="expp", bufs=4) as expp, \
                 tc.tile_pool(name="drp", bufs=2) as drp, \
                 tc.tile_pool(name="rscp", bufs=2) as rscp, \
                 tc.tile_pool(name="scps", bufs=2, space="PSUM") as scps, \
                 tc.tile_pool(name="avps", bufs=2, space="PSUM") as avps:

                def emit_av(av, ex, k, last):
                    for h in range(4):
                        colo = (h % 2) * 512 + (h // 2) * 256
                        nc.tensor.matmul(
                            av[0:65, colo:colo + 256],
                            v_aug[:, h, k * 65:k * 65 + 65],
                            ex[:, colo:colo + 256],
                            start=(k == 0 and h < 2), stop=last,
                            skip_group_check=True)

                def epi_dve(qg, av):
                    avsb = drp.tile([P, 1024], bf16, name=f"avsb{qg}",
                                    tag="avsb")
                    nc.vector.tensor_copy(avsb[0:65, :], av[0:65, :])
                    denr = drp.tile([1, 1024], bf16, name=f"denr{qg}",
                                    tag="denr")
                    with nc.allow_low_precision(reason="softmax denom bf16"):
                        nc.vector.reciprocal(denr[:], avsb[64:65, :])
                    return avsb, denr

                def epi_pe(qg, av):
                    avsb, denr = epi_dve(qg, av)
                    qsl = slice(qg * 256, (qg + 1) * 256)
                    for h in range(4):
                        colo = (h % 2) * 512 + (h // 2) * 256
                        nc.tensor.matmul(
                            av[64:128, colo:colo + 256],
                            onesb[0:1, 0:64], denr[0:1, colo:colo + 256],
                            start=True, stop=True, skip_group_check=True)
                    for h in range(4):
                        hb = (h % 2) * 64
                        colo = (h % 2) * 512 + (h // 2) * 256
                        nc.vector.tensor_mul(
                            attnT[hb:hb + 64, h // 2, qsl],
                            avsb[0:64, colo:colo + 256],
                            av[64:128, colo:colo + 256])

                def emit_proj(c, tt):
                    # proj partial, token-major rows of rs_in[c]
                    qg0 = CH[c][0]
                    pp = scps.tile([P, 1024], f32, name=f"pp{c}_{tt}", tag="sc")
                    gt0 = qg0 * 256 + tt * P
                    if F8Q:
                        for oc in range(2):
                            nc.tensor.matmul(
                                pp[:, oc * 512:(oc + 1) * 512],
                                attnT[:, 0:2, gt0:gt0 + P],
                                wp[:, 0:2, oc * 512:(oc + 1) * 512],
                                start=True, stop=True,
                                perf_mode=mybir.MatmulPerfMode.DoubleRow)
                    else:
                        for oc in range(2):
                            for ct in range(2):
                                nc.tensor.matmul(
                                    pp[:, oc * 512:(oc + 1) * 512],
                                    attnT[:, ct, gt0:gt0 + P],
                                    wp[:, ct, oc * 512:(oc + 1) * 512],
                                    start=(ct == 0), stop=(ct == 1))
                    rsct = rscp.tile([P, C], bf16, name=f"rsc{c}_{tt}",
                                     tag="rsc")
                    nc.scalar.mul(rsct[:], pp[:], 1.0 / _QS)
                    nc.sync.dma_start(rs_in[c][tt * P:(tt + 1) * P, :], rsct[:])

                def send_rs(c):
                    nc.gpsimd.collective_compute(
                        "ReduceScatter", mybir.AluOpType.add,
                        ins=[rs_in[c].opt()], outs=[rs_out[c].opt()],
                        replica_groups=[[0, 1, 2, 3], [4, 5, 6, 7]])

                pend = None        # (qg, av, avsb, denr) awaiting PE epilogue
                projq = []         # pp tiles awaiting emission
                emitted = [0, 0, 0, 0]

                def flush_projq():
                    for c_, tt_ in projq:
                        emit_proj(c_, tt_)
                        emitted[c_] += 1
                        if emitted[c_] == CH[c_][4] // P:
                            send_rs(c_)
                    projq.clear()

                for qg in range(8):
                    K = 2 * qg + 2
                    qsl = slice(qg * 256, (qg + 1) * 256)
                    av = avps.tile([P, 1024], f32, name=f"av{qg}", tag="av")
                    if qg == 5:
                        emit_ln2_head_a(0)
                    elif qg == 6:
                        emit_ln2_head(0, scps)
                        emit_ln2_head_a(1)
                    elif qg == 7:
                        emit_ln2_head(1, scps)
                    prev = None
                    for k in range(K):
                        sc = scps.tile([P, 1024], f32, name=f"sc{qg}_{k}",
                                       tag="sc")
                        for h in range(4):
                            hb = (h % 2) * 64
                            colo = (h % 2) * 512 + (h // 2) * 256
                            nc.tensor.matmul(
                                sc[:, colo:colo + 256],
                                kT[hb:hb + 64, h // 2, k * P:(k + 1) * P],
                                qT[hb:hb + 64, h // 2, qsl],
                                start=True, stop=True)
                        ex = expp.tile([P, 1024], bf16, name=f"ex{qg}_{k}",
                                       tag="ex")
                        nc.scalar.activation(ex[:], sc[:], AF.Exp)
                        if k == 2 * qg:
                            nc.vector.tensor_mul(ex[:], ex[:], maskA[:])
                        elif k == 2 * qg + 1:
                            nc.vector.tensor_mul(ex[:], ex[:], maskB[:])
                        if k == 2 and pend is not None:
                            epi_pe(*pend)      # prev qg's normalization
                            pend = None
                        if prev is not None:
                            emit_av(av, prev[0], prev[1], False)
                        prev = (ex, k)
                    emit_av(av, prev[0], prev[1], True)
                    if qg <= 3:
                        epi_pe(qg, av)       # eager: un-gates early RS
                        pend = None
                    else:
                        pend = (qg, av)
                    flush_projq()
                    # queue this qg's proj tiles (need epi_pe of this qg)
                    for c in range(4):
                        if CH[c][0] <= qg < CH[c][1]:
                            b0 = (qg - CH[c][0]) * 2
                            projq += [(c, b0), (c, b0 + 1)]
                    if qg in (1, 3):
                        flush_projq()        # eager: un-gates early RS
                    # collectives fire once their chunk's proj is queued+done
                epi_pe(*pend)
                flush_projq()

        # =========== phase 2: MLP per chunk ===========
        with tc.tile_pool(name="mlp", bufs=1) as mlp, \
             tc.tile_pool(name="outp", bufs=1) as outp, \
             tc.tile_pool(name="mps", bufs=2, space="PSUM") as mps, \
             tc.tile_pool(name="ups", bufs=2, space="PSUM") as ups, \
             tc.tile_pool(name="dps", bufs=2, space="PSUM") as dps:
            outdst = d["OUT"].rearrange("(ot p) t -> p ot t", p=P)
            wdown_b = mlp.tile([P, 16, C], _MDT, name="wdown_b")
            nc.sync.dma_start(wdown_b[:], wdsrc[:, 16:32, :])
            for c in range(4):
                _, _, co, w, _ = CH[c]
                csl = slice(co, co + w)
                if c == 2:
                    emit_ln2_head_a(2)
                    emit_ln2_head(2, mps)
                # ---- up + gelu ----
                hT = mlp.tile([P, NFT, w], _MDT, name=f"hT{c}", tag="hT")
                nsub = 512 // w
                for fg in range(NFT // nsub):
                    pu = ups.tile([P, 512], f32, name=f"pu{c}_{fg}", tag="pu")
                    for sub in range(nsub):
                        ft = fg * nsub + sub
                        if F8M:
                            for c2 in range(NCT // 2):
                                nc.tensor.matmul(
                                    pu[:, sub * w:(sub + 1) * w],
                                    wup[:, 2 * c2:2 * c2 + 2,
                                        ft * P:(ft + 1) * P],
                                    g2[c][:, 2 * c2:2 * c2 + 2, :],
                                    start=(c2 == 0 and sub == 0),
                                    stop=(c2 == NCT // 2 - 1),
                                    perf_mode=mybir.MatmulPerfMode.DoubleRow,
                                    skip_group_check=True)
                        else:
                            for ct in range(NCT):
                                nc.tensor.matmul(
                                    pu[:, sub * w:(sub + 1) * w],
                                    wup[:, ct, ft * P:(ft + 1) * P],
                                    g2[c][:, ct, :],
                                    start=(ct == 0 and sub == 0),
                                    stop=(ct == NCT - 1),
                                    skip_group_check=True)
                    for sub in range(nsub):
                        ft = fg * nsub + sub
                        nc.scalar.activation(
                            hT[:, ft, :], pu[:, sub * w:(sub + 1) * w],
                            AF.Gelu, bias=bup[:, ft:ft + 1], scale=1.0 / _MS)
                # ---- down + bias + residual ----
                for ot in range(8):
                    pd = dps.tile([P, 256], f32, name=f"pd{c}_{ot}", tag="pd")
                    if F8M:
                        for f2 in range(NFT // 2):
                            cf = 2 * f2
                            wd, ci = ((wdown_a, cf) if cf < 16
                                      else (wdown_b, cf - 16))
                            nc.tensor.matmul(
                                pd[:, 0:w],
                                wd[:, ci:ci + 2, ot * P:(ot + 1) * P],
                                hT[:, cf:cf + 2, :],
                                start=(f2 == 0), stop=(f2 == NFT // 2 - 1),
                                perf_mode=mybir.MatmulPerfMode.DoubleRow)
                    else:
                        for cf in range(NFT):
                            wd, ci = ((wdown_a, cf) if cf < 16
                                      else (wdown_b, cf - 16))
                            nc.tensor.matmul(
                                pd[:, 0:w], wd[:, ci, ot * P:(ot + 1) * P],
                                hT[:, cf, :],
                                start=(cf == 0), stop=(cf == NFT - 1))
                    td = outp.tile([P, 256], f32, name=f"td{c}_{ot}", tag="td",
                                   bufs=2)
                    nc.scalar.activation(td[:, 0:w], pd[:, 0:w], AF.Identity,
                                         bias=bdown[:, ot:ot + 1],
                                         scale=1.0 / _MS)
                    outO = outp.tile([P, 256], f32, name=f"outO{c}_{ot}",
                                     tag="outO", bufs=2)
                    nc.vector.tensor_add(outO[:, 0:w], td[:, 0:w],
                                         x1T[:, ot, csl])
                    nc.sync.dma_start(outdst[:, ot, csl], outO[:, 0:w])
                    if c == 2 and ot == 0:
                        emit_ln2_head_a(3)

                    if c == 2 and ot == 2:
                        emit_ln2_head(3, mps)


def _prep_inputs(x, ln1_w, ln1_b, c_attn_w, c_attn_b, c_proj_w, c_proj_b,
                 ln2_w, ln2_b, up_w, up_b, down_w, down_b):
    """Host-side preprocessing -> list of 8 per-core input dicts."""
    x = np.asarray(x, np.float32)
    f64 = np.float64
    mu = x.mean(-1, keepdims=True, dtype=f64)
    var = np.asarray(x, f64).var(-1, keepdims=True)
    g = ((x - mu) / np.sqrt(var + EPS)).astype(np.float32)     # [B, T, C]

    ln1_w = np.asarray(ln1_w, np.float32); ln1_b = np.asarray(ln1_b, np.float32)
    ln2_w = np.asarray(ln2_w, np.float32); ln2_b = np.asarray(ln2_b, np.float32)
    c_attn_w = np.asarray(c_attn_w, np.float32)
    c_attn_b = np.asarray(c_attn_b, np.float32)
    c_proj_w = np.asarray(c_proj_w, np.float32)
    c_proj_b = np.asarray(c_proj_b, np.float32)
    up_w = np.asarray(up_w, np.float32); up_b = np.asarray(up_b, np.float32)
    down_w = np.asarray(down_w, np.float32)
    down_b = np.asarray(down_b, np.float32)

    Wa = c_attn_w * ln1_w[None, :]
    ba = c_attn_b + c_attn_w @ ln1_b
    Wq, Wk, Wv = Wa[:C], Wa[C:2 * C], Wa[2 * C:]
    bqv, bkv, bvv = ba[:C], ba[C:2 * C], ba[2 * C:]
    s = 1.0 / np.sqrt(D)
    Wq = Wq * s; bqv = bqv * s

    Wup = up_w * ln2_w[None, :]
    bupv = up_b + up_w @ ln2_b

    def b2t(v, n):   # per-partition bias layout [128, n]
        return np.ascontiguousarray(v.reshape(n, P).T.astype(np.float32))

    # diag-tile masks [128 kv, 256 q] tiled x4 heads
    tk = np.arange(P)[:, None]
    tq = np.arange(P)[None, :]
    mA = (tk <= tq).astype(np.float32)
    blockA = np.concatenate([mA, np.ones((P, P), np.float32)], axis=1)
    blockB = np.concatenate([np.zeros((P, P), np.float32), mA], axis=1)
    maskA = np.tile(blockA, (1, 4)).astype(BF)
    maskB = np.tile(blockB, (1, 4)).astype(BF)

    shared = {
        "WupT": np.ascontiguousarray(Wup.T * _MS).astype(_MNP),
        "WdownT": np.ascontiguousarray(down_w.T * _MS).astype(_MNP),
        "bup": b2t(bupv, 32), "bdown": b2t(down_b, 8),
        "maskA": maskA, "maskB": maskB,
    }

    xb = x + c_proj_b[None, None, :]
    in_maps = []
    for core in range(8):
        b, j = core // 4, core % 4
        hsl = slice(256 * j, 256 * j + 256)
        m = dict(shared)
        m["gT"] = np.ascontiguousarray(g[b].T).astype(_QNP)
        m["WqT"] = np.ascontiguousarray(Wq[hsl].T * _QS).astype(_QNP)
        m["WkT"] = np.ascontiguousarray(Wk[hsl].T * _QS).astype(_QNP)
        m["WvT"] = np.ascontiguousarray(Wv[hsl].T * _QS).astype(_QNP)
        m["WpT"] = np.ascontiguousarray(c_proj_w[:, hsl].T * _QS).astype(_QNP)
        m["bq"] = b2t(bqv[hsl], 2)
        m["bk"] = b2t(bkv[hsl], 2)
        m["brep"] = np.broadcast_to(bvv[hsl].astype(BF), (P, 256)).copy()
        cols = np.r_[128 * j:128 * j + 128,
                     512 + 128 * j:512 + 128 * j + 128,
                     1024 + 128 * j:1024 + 128 * j + 128,
                     1536 + 128 * j:1536 + 128 * j + 128]
        m["xbT"] = np.ascontiguousarray(xb[b].T[:, cols]).astype(BF)
        in_maps.append(m)
    return in_maps


def kernel(**inputs):
    global _CACHED_NC
    if _CACHED_NC is None:
        _CACHED_NC = _build_nc()
    nc = _CACHED_NC
    in_maps = _prep_inputs(**inputs)
    try:
        res = run_bass_kernel_spmd(nc, in_maps, list(range(8)))
    except Exception:
        res = run_bass_kernel_spmd(nc, in_maps, list(range(8)))
    out = np.empty((B, T, C), np.float32)
    for core in range(8):
        o = res.results[core]["OUT"]                # [C, 512]
        b, j = core // 4, core % 4
        out[b, 128 * j:128 * j + 128, :] = o[:, 0:128].T
        out[b, 512 + 128 * j:512 + 128 * j + 128, :] = o[:, 128:256].T
        out[b, 1024 + 128 * j:1024 + 128 * j + 128, :] = o[:, 256:384].T
        out[b, 1536 + 128 * j:1536 + 128 * j + 128, :] = o[:, 384:512].T
    return out
